# revision 10
# baseline (speedup 1.0000x reference)
"""Farthest-point-sampling (npoint=2) Bass kernel for Trainium2 — v4.

Problem: xyz [1, 64, 3, 262144] fp32 -> indices [64, 2] (int64 on host).
Per batch b:
  idx0 = argmax_n y[n]            (y = coord plane 1)
  c    = (x,y,z)[idx0]
  idx1 = argmax_n ((x-cx)^2 + (y-cy)^2 + (z-cz)^2)
argmax = first occurrence on ties (jnp.argmax semantics).

Sharding: data-parallel over batch; 8 NeuronCores x 8 batches each.

v4 structure (per core, 8 batches; plane viewed as [128, 2048] fp32):
  * argmax via chunk hierarchy: ONE VectorE grouped tensor_reduce
    [128,(16,128)] -> chunk maxima [128,16]; a tiny transpose-finale
    locates the global max chunk (pk code, first-occurrence order), an
    indirect-DMA gather pulls the 128-wide window back from HBM and a
    tiny find locates the column.
  * distances: ScalarE squares (v + (-c))^2 (bit-exact vs reference);
    GpSimd adds sqx+sqy in halves; VectorE add + grouped chunk-max-reduce
    per half.  Dist finale mirrors the y finale (128-wide windows,
    exact recompute of the window with the same fp32 add order).
  * y finales are split at the gather boundary and interleaved with the
    next group's reduces so the in-order V queue never sits in a
    cross-engine round trip; phase_b trails two groups behind.
  * DMA issue order: y0,y1,x0,z0,y2,y3,x1,z1,... keeps the y stream
    (which gates the centroid) ahead of x/z.
"""

import numpy as np

import concourse.bacc as bacc
import concourse.bass as bass
import concourse.mybir as mybir
from concourse.masks import make_identity
from concourse.tile import TileContext

B = 64  # full batch
N_CORES = 8
BPC = B // N_CORES  # batches per core
N = 262144
P = 128
COLS = N // P  # 2048
K = 16  # chunks per partition row
J = 128  # columns per chunk
KBIG = 4096.0  # > P*K codes
ROWJ = 3 * N // J  # 6144 rows of 128 per batch in the row view
H = COLS // 2  # half-plane columns
NEG_INF = -3.4e38

YG = 2  # y-finale group size
NG = BPC // YG

F32 = mybir.dt.float32
U32 = mybir.dt.uint32
I32 = mybir.dt.int32
AX = mybir.AxisListType.X
OP = mybir.AluOpType
SQUARE = mybir.ActivationFunctionType.Square


def build_nc():
    nc = bacc.Bacc()
    xin = nc.dram_tensor("xyz", [BPC, 3, N], F32, kind="ExternalInput")
    out = nc.dram_tensor("idx", [1, 2 * BPC], I32, kind="ExternalOutput")

    xin_rows = xin.rearrange("b c (r j) -> (b c r) j", j=J)
    xin_flat = xin.rearrange("b c n -> (b c n)")[:, None]

    with TileContext(nc) as tc:
        with (
            tc.tile_pool(name="consts", bufs=1) as consts,
            tc.tile_pool(name="ypool", bufs=BPC) as ypool,
            tc.tile_pool(name="xz", bufs=3) as xz,
            tc.tile_pool(name="sq", bufs=2) as sqp,
            tc.tile_pool(name="acc", bufs=1) as acc,
            tc.tile_pool(name="small", bufs=2) as small,
            tc.tile_pool(name="fin", bufs=4) as fin,
            tc.tile_pool(name="pt", bufs=3, space="PSUM") as ptp,
        ):
            # ---------- first y DMAs go out before const setup ----------
            tys = [
                ypool.tile([P, COLS], F32, tag="ty", name=f"ty{b}")
                for b in range(BPC)
            ]
            for b in range(BPC):
                nc.sync.dma_start(
                    tys[b], xin[b, 1].rearrange("(p m) -> p m", p=P)
                )

            # ---------- constants ----------
            ident = consts.tile([P, P], F32)
            make_identity(nc, ident)
            ones = consts.tile([1, P], F32)
            nc.vector.memset(ones, 1.0)

            def iota_f32(shape, pattern, base, mult, name):
                t_i = consts.tile(shape, I32, tag="stage", name=name + "i")
                nc.gpsimd.iota(
                    t_i, pattern=pattern, base=base, channel_multiplier=mult
                )
                t_f = consts.tile(shape, F32, name=name + "f")
                nc.vector.tensor_copy(t_f, t_i)
                return t_f

            # revk[p, b*K + k] = KBIG - (p*K + k)
            revk = iota_f32([P, P], [[0, BPC], [-1, K]], int(KBIG), -K, "revk")
            # revj[b, j] = J - j
            revj = iota_f32([BPC, J], [[-1, J]], J, 0, "revj")
            # y window rows (per group): KBIG + b*ROWJ + 2048
            rowy_g = [
                iota_f32(
                    [YG, 1], [[0, 1]], int(KBIG) + g0 * ROWJ + 2048, ROWJ,
                    f"rowy{g0}",
                )
                for g0 in range(0, BPC, YG)
            ]
            # centroid element-gather bases: 524416 + b*3N (+2N for z)
            exbx_g = [
                iota_f32(
                    [YG, 1], [[0, 1]], 524416 + g0 * 3 * N, 3 * N, f"exbx{g0}"
                )
                for g0 in range(0, BPC, YG)
            ]
            exbz_g = [
                iota_f32(
                    [YG, 1], [[0, 1]], 524416 + g0 * 3 * N + 2 * N, 3 * N,
                    f"exbz{g0}",
                )
                for g0 in range(0, BPC, YG)
            ]
            # dist window rows (all 8 batches, per plane c)
            rowd_c = [
                iota_f32(
                    [BPC, 1], [[0, 1]], int(KBIG) + c * COLS, ROWJ, f"rowd{c}"
                )
                for c in range(3)
            ]

            # ---------- accumulators ----------
            Myall = acc.tile([P, P], F32)   # y chunk maxima [p, b*16+k]
            Mdall = acc.tile([P, P], F32)   # dist chunk maxima
            nbx = acc.tile([P, BPC], F32)   # -cx bias columns
            nby = acc.tile([P, BPC], F32)
            nbz = acc.tile([P, BPC], F32)
            out_i = acc.tile([1, 2 * BPC], I32)

            def argmax_head(Mall, b0, g, tag):
                """chunk maxima -> (gm [g,1], best [g,1] = KBIG - pk)."""
                mg = small.tile([P, g], F32, tag="mg", name=f"mg{tag}")
                nc.vector.tensor_reduce(
                    mg,
                    Mall[:, K * b0 : K * (b0 + g)].rearrange(
                        "p (g k) -> p g k", k=K
                    ),
                    axis=AX, op=OP.max,
                )
                pmg = ptp.tile([g, P], F32, tag="pt", name=f"pmg{tag}")
                nc.tensor.transpose(pmg, mg, ident)
                gm = small.tile([g, 1], F32, tag="gm", name=f"gm{tag}")
                nc.vector.tensor_reduce(gm, pmg, axis=AX, op=OP.max)
                pgt = ptp.tile([1, g], F32, tag="pt", name=f"pgt{tag}")
                nc.tensor.transpose(pgt, gm, ident[0:g, 0:g])
                gmr = small.tile([1, g], F32, tag="gmr", name=f"gmr{tag}")
                nc.vector.tensor_copy(gmr, pgt)
                pgb = ptp.tile([P, g], F32, tag="pt", name=f"pgb{tag}")
                nc.tensor.matmul(pgb, ones, gmr, start=True, stop=True)
                gmb = small.tile([P, g], F32, tag="gmb", name=f"gmb{tag}")
                nc.vector.tensor_copy(gmb, pgb)
                cand = small.tile([P, g * K], F32, tag="cand", name=f"cand{tag}")
                for i in range(g):
                    nc.vector.scalar_tensor_tensor(
                        out=cand[:, K * i : K * (i + 1)],
                        in0=Mall[:, K * (b0 + i) : K * (b0 + i + 1)],
                        scalar=gmb[:, i : i + 1],
                        in1=revk[:, K * (b0 + i) : K * (b0 + i + 1)],
                        op0=OP.is_equal, op1=OP.mult,
                    )
                cred = small.tile([P, g], F32, tag="cred", name=f"cred{tag}")
                nc.vector.tensor_reduce(
                    cred, cand.rearrange("p (g k) -> p g k", k=K),
                    axis=AX, op=OP.max,
                )
                pcr = ptp.tile([g, P], F32, tag="pt", name=f"pcr{tag}")
                nc.tensor.transpose(pcr, cred, ident)
                best = small.tile([g, 1], F32, tag="best", name=f"best{tag}")
                nc.vector.tensor_reduce(best, pcr, axis=AX, op=OP.max)
                return gm, best

            def gather_win(best, rowconst, g, tag):
                rowu = small.tile([g, 1], U32, tag="rowu", name=f"rowu{tag}")
                nc.vector.tensor_scalar(
                    out=rowu, in0=best, scalar1=-1.0, scalar2=rowconst,
                    op0=OP.mult, op1=OP.add,
                )
                win = fin.tile([g, J], F32, tag="win", bufs=4, name=f"win{tag}")
                nc.gpsimd.indirect_dma_start(
                    out=win, out_offset=None, in_=xin_rows,
                    in_offset=bass.IndirectOffsetOnAxis(ap=rowu[0:g, 0:1], axis=0),
                )
                return win

            def window_find(win, gm, g, tag):
                wc = fin.tile([g, J], F32, tag="fscr", name=f"wc{tag}")
                nc.vector.scalar_tensor_tensor(
                    out=wc, in0=win, scalar=gm, in1=revj[0:g, :],
                    op0=OP.is_equal, op1=OP.mult,
                )
                wbest = small.tile([g, 1], F32, tag="wb", name=f"wb{tag}")
                nc.vector.tensor_reduce(wbest, wc, axis=AX, op=OP.max)
                return wbest

            def emit_idx(best, wbest, g, out_cols, tag):
                """idx = 524416 - 128*best - wbest -> out_i (i32); returns q."""
                q = small.tile([g, 1], F32, tag="q", name=f"q{tag}")
                nc.vector.scalar_tensor_tensor(
                    out=q, in0=best, scalar=-128.0, in1=wbest,
                    op0=OP.mult, op1=OP.subtract,
                )
                idxf = small.tile([g, 1], F32, tag="idxf", name=f"idxf{tag}")
                nc.vector.tensor_scalar(
                    out=idxf, in0=q, scalar1=1.0, scalar2=524416.0,
                    op0=OP.mult, op1=OP.add,
                )
                pidx = ptp.tile([1, g], F32, tag="pt", name=f"pidx{tag}")
                nc.tensor.transpose(pidx, idxf, ident[0:g, 0:g])
                nc.scalar.copy(out_i[0:1, out_cols], pidx)
                return q

            def bias_cols(vals, g, b0, dst, tag):
                """dst[:, b0:b0+g] = -vals broadcast down all 128 rows."""
                pv = ptp.tile([1, g], F32, tag="pt", name=f"pv{tag}")
                nc.tensor.transpose(pv, vals, ident[0:g, 0:g])
                nrow = small.tile([1, g], F32, tag="nrow", name=f"nrow{tag}")
                nc.scalar.mul(nrow, pv, -1.0)
                pb = ptp.tile([P, g], F32, tag="pt", name=f"pb{tag}")
                nc.tensor.matmul(pb, ones, nrow, start=True, stop=True)
                nc.scalar.copy(dst[:, b0 : b0 + g], pb)

            # --- y finale, split at the gather boundary ---
            fstate = {}

            def y_finale_head(g0):
                g = YG
                gm, best = argmax_head(Myall, g0, g, f"y{g0}")
                win = gather_win(best, rowy_g[g0 // YG], g, f"y{g0}")
                fstate[g0] = (gm, best, win)

            def y_finale_tail(g0):
                g = YG
                gm, best, win = fstate.pop(g0)
                wbest = window_find(win, gm, g, f"y{g0}")
                q = emit_idx(best, wbest, g, slice(g0, g0 + g), f"y{g0}")
                offx = small.tile([g, 1], U32, tag="offx", name=f"offx{g0}")
                nc.vector.tensor_scalar(
                    out=offx, in0=q, scalar1=1.0, scalar2=exbx_g[g0 // YG],
                    op0=OP.mult, op1=OP.add,
                )
                offz = small.tile([g, 1], U32, tag="offz", name=f"offz{g0}")
                nc.vector.tensor_scalar(
                    out=offz, in0=q, scalar1=1.0, scalar2=exbz_g[g0 // YG],
                    op0=OP.mult, op1=OP.add,
                )
                cx = small.tile([g, 1], F32, tag="cx", name=f"cx{g0}")
                nc.gpsimd.indirect_dma_start(
                    out=cx, out_offset=None, in_=xin_flat,
                    in_offset=bass.IndirectOffsetOnAxis(ap=offx[0:g, 0:1], axis=0),
                )
                cz = small.tile([g, 1], F32, tag="cz", name=f"cz{g0}")
                nc.gpsimd.indirect_dma_start(
                    out=cz, out_offset=None, in_=xin_flat,
                    in_offset=bass.IndirectOffsetOnAxis(ap=offz[0:g, 0:1], axis=0),
                )
                bias_cols(cx, g, g0, nbx, f"bx{g0}")
                bias_cols(gm, g, g0, nby, f"by{g0}")  # cy == max y value
                bias_cols(cz, g, g0, nbz, f"bz{g0}")

            txs, tzs = {}, {}

            def issue_xz(b):
                tx = xz.tile([P, COLS], F32, tag="tx", name=f"tx{b}")
                nc.scalar.dma_start(tx, xin[b, 0].rearrange("(p m) -> p m", p=P))
                tz = xz.tile([P, COLS], F32, tag="tz", name=f"tz{b}")
                nc.scalar.dma_start(tz, xin[b, 2].rearrange("(p m) -> p m", p=P))
                txs[b], tzs[b] = tx, tz

            def phase_b(b):
                """S: 3 exact squares; G: sqx+sqy halves; V: (+sqz) add then
                grouped chunk-max into Mdall[:, 16b:16b+16]."""
                sqx = sqp.tile([P, COLS], F32, tag="sqx", name=f"sqx{b}")
                nc.scalar.activation(sqx, txs[b], SQUARE, bias=nbx[:, b : b + 1])
                sqy = sqp.tile([P, COLS], F32, tag="sqy", name=f"sqy{b}")
                nc.scalar.activation(sqy, tys[b], SQUARE, bias=nby[:, b : b + 1])
                sqz = sqp.tile([P, COLS], F32, tag="sqz", name=f"sqz{b}")
                nc.scalar.activation(sqz, tzs[b], SQUARE, bias=nbz[:, b : b + 1])
                for h in range(2):
                    lo, hi = h * H, (h + 1) * H
                    s1 = sqp.tile([P, H], F32, tag="s1", name=f"s1_{b}_{h}")
                    nc.gpsimd.tensor_add(s1, sqx[:, lo:hi], sqy[:, lo:hi])
                    s2 = sqp.tile([P, H], F32, tag="s2", name=f"s2_{b}_{h}")
                    nc.vector.tensor_add(s2, s1, sqz[:, lo:hi])
                    nc.vector.tensor_reduce(
                        Mdall[:, K * b + h * (K // 2) : K * b + (h + 1) * (K // 2)],
                        s2.rearrange("p (k j) -> p k j", j=J),
                        axis=AX, op=OP.max,
                    )

            def chunk_red_y(b):
                nc.vector.tensor_reduce(
                    Myall[:, K * b : K * (b + 1)],
                    tys[b].rearrange("p (k j) -> p k j", j=J),
                    axis=AX, op=OP.max,
                )

            # ---------- interleaved emission ----------
            # iter i: y DMAs(i), y reduces(i), finale-tail(i-1),
            # finale-head(i), xz(i), phase_b(group i-2)
            for i in range(NG):
                b0 = YG * i
                chunk_red_y(b0)
                chunk_red_y(b0 + 1)
                if i > 0:
                    y_finale_tail(YG * (i - 1))
                y_finale_head(b0)
                issue_xz(i)
                if i > 1:
                    phase_b(YG * (i - 2))
                    phase_b(YG * (i - 2) + 1)
            y_finale_tail(YG * (NG - 1))
            for b in range(NG, BPC):
                issue_xz(b)
            for b in range(YG * (NG - 2), BPC):
                phase_b(b)

            # ---------- dist finale (batched over all 8) ----------
            gm_d, best_d = argmax_head(Mdall, 0, BPC, "d")
            negc8 = []
            for name, nb in (("x", nbx), ("y", nby), ("z", nbz)):
                dtmp = small.tile([BPC, BPC], F32, tag="dg", name=f"dg{name}")
                nc.vector.tensor_tensor(
                    dtmp, nb[0:BPC, :], ident[0:BPC, 0:BPC], op=OP.mult
                )
                dneg = small.tile([BPC, 1], F32, tag="dn", name=f"dn{name}")
                nc.vector.tensor_reduce(dneg, dtmp, axis=AX, op=OP.add)
                negc8.append(dneg)
            wins = [
                gather_win(best_d, rowd_c[c], BPC, f"d{c}") for c in range(3)
            ]
            wsq = []
            for c in range(3):
                s = fin.tile([BPC, J], F32, tag="fscr", name=f"wsq{c}")
                nc.scalar.activation(s, wins[c], SQUARE, bias=negc8[c])
                wsq.append(s)
            wd1 = fin.tile([BPC, J], F32, tag="fscr", name="wd1")
            nc.vector.tensor_add(wd1, wsq[0], wsq[1])
            wd2 = fin.tile([BPC, J], F32, tag="fscr", name="wd2")
            nc.vector.tensor_add(wd2, wd1, wsq[2])
            wbest_d = window_find(wd2, gm_d, BPC, "d")
            emit_idx(best_d, wbest_d, BPC, slice(BPC, 2 * BPC), "d")

            nc.sync.dma_start(out[:, :], out_i[:, :])

    nc.compile()
    return nc


_NC_CACHE = None


def _get_nc():
    global _NC_CACHE
    if _NC_CACHE is None:
        _NC_CACHE = build_nc()
    return _NC_CACHE


def kernel(xyz: np.ndarray) -> np.ndarray:
    from concourse.bass_utils import run_bass_kernel_spmd

    assert xyz.shape == (1, B, 3, N), xyz.shape
    xyz = np.ascontiguousarray(xyz, dtype=np.float32)
    nc = _get_nc()
    in_maps = [
        {"xyz": np.ascontiguousarray(xyz[0, k * BPC : (k + 1) * BPC])}
        for k in range(N_CORES)
    ]
    res = run_bass_kernel_spmd(nc, in_maps, core_ids=list(range(N_CORES)))
    # out layout per core: [1, 16] = [idx0 x8 | idx1 x8]
    outs = [res.results[k]["idx"].reshape(2, BPC).T for k in range(N_CORES)]
    return np.concatenate(outs, axis=0).astype(np.int64)
